# revision 1
# baseline (speedup 1.0000x reference)
"""CrossAttention kernel for Trainium2, 8 NeuronCores.

Reference pipeline (B=4, C=256, H=W=64, N=4096, d=C//8=32):
  sub = x1 - x2
  x3 = relu(bn1(pw1(dw1([sub, x1]))))      # dw: 3x3 grouped conv (groups=C)
  x4 = relu(bn2(pw2(dw2([sub, x2]))))      # pw: 1x1 512->256
  q = wq@x4 [B,32,N]; k = wk@x3 [B,32,N]; v = wv@x3 [B,256,N]
  attn = softmax(q^T k);  out = gamma * (v @ attn^T) + x1

Sharding: 8 cores = (batch b) x (pixel-half h). Each core computes BOTH
conv paths only for its own pixel half (with a one-row halo), projects
k / v^T / q from its half, then the pair exchanges k and v^T via
AllGather so each core can run flash attention for its 2048 queries over
all 4096 keys.

Device-side choices:
  - dw conv on the PE as 9 accumulating block-diagonal [128x128] matmuls
    over a zero-padded 66-col image layout (8-row windows = 512 output
    columns via a strided rhs AP that skips the pad columns).
  - energy is computed transposed, E^T[j, i]; the softmax denominator
    comes for free from an appended ones-column in v^T in the second
    (accumulating) matmul.
  - matmul datapath in bf16 (fp32 PSUM accumulation); normalize /
    transpose / residual-add in fp32.
  - gamma folded into wv/bv on the host; bn+biases folded into per-
    channel scale/shift applied by ScalarE during PSUM eviction.
"""

import numpy as np
import ml_dtypes

import concourse.bass as bass
import concourse.mybir as mybir
import concourse.tile as tile
from concourse import bacc
from concourse.bass_utils import run_bass_kernel_spmd

F32 = mybir.dt.float32
F32R = mybir.dt.float32r
BF16 = mybir.dt.bfloat16
AF = mybir.ActivationFunctionType
ALU = mybir.AluOpType

B, C, H, W = 4, 256, 64, 64
N = H * W            # 4096 pixels
QH = N // 2          # pixels per core (queries/own keys)
EPS = 1e-5
PW = 66              # padded row width
OFF = 2              # leading pad elements in padded tiles
SLOTS = 34           # 32 data rows + halo/zero rows
CAT_F = OFF + SLOTS * PW + OFF   # 2248
VT = 258             # v^T row: 256 channels + ones + pad
PAIRS = [[0, 1], [2, 3], [4, 5], [6, 7]]

_CACHE = {}


def _build_nc():
    nc = bacc.Bacc("TRN2", target_bir_lowering=False, debug=False, num_devices=8)

    cat1p = nc.dram_tensor("cat1p", [4, 128, CAT_F], BF16, kind="ExternalInput")
    cat2p = nc.dram_tensor("cat2p", [4, 128, CAT_F], BF16, kind="ExternalInput")
    x1h_d = nc.dram_tensor("x1h", [2, 128, QH], F32, kind="ExternalInput")
    w1bd = nc.dram_tensor("w1bd", [4, 128, 9 * 128], BF16, kind="ExternalInput")
    w2bd = nc.dram_tensor("w2bd", [4, 128, 9 * 128], BF16, kind="ExternalInput")
    pw1T = nc.dram_tensor("pw1T", [4, 128, 256], BF16, kind="ExternalInput")
    pw2T = nc.dram_tensor("pw2T", [4, 128, 256], BF16, kind="ExternalInput")
    wvT = nc.dram_tensor("wvT", [2, 128, 256], BF16, kind="ExternalInput")
    wkT = nc.dram_tensor("wkT", [2, 128, 32], BF16, kind="ExternalInput")
    wqT = nc.dram_tensor("wqT", [2, 128, 32], BF16, kind="ExternalInput")
    bn1_d = nc.dram_tensor("bn1", [128, 4], F32, kind="ExternalInput")
    bn2_d = nc.dram_tensor("bn2", [128, 4], F32, kind="ExternalInput")
    bkq_d = nc.dram_tensor("bkq", [128, 1], F32, kind="ExternalInput")
    bvg_d = nc.dram_tensor("bvg", [2, 128, 1], F32, kind="ExternalInput")
    ident_d = nc.dram_tensor("ident", [128, 128], F32R, kind="ExternalInput")
    vinit_d = nc.dram_tensor("vinit", [128, 16 * VT], BF16, kind="ExternalInput")
    out_d = nc.dram_tensor("out", [2, 128, QH], F32, kind="ExternalOutput")

    # collective bounce buffers
    kown_d = nc.dram_tensor("kown_b", [32, QH], BF16)
    kfull_d = nc.dram_tensor("kfull_b", [64, QH], BF16)
    vown_d = nc.dram_tensor("vown_b", [128, 16 * VT], BF16)
    vfull_d = nc.dram_tensor("vfull_b", [2, 128, 16 * VT], BF16)

    with tile.TileContext(nc) as tc:
        with tc.tile_pool(name="persist", bufs=1) as pp:
            x3o = [pp.tile([128, QH], BF16, name=f"x3o_{m}", tag=f"x3o_{m}")
                   for m in range(2)]
            x4 = [pp.tile([128, QH], BF16, name=f"x4_{m}", tag=f"x4_{m}")
                  for m in range(2)]
            bn1 = pp.tile([128, 4], F32, name="bn1", tag="bn1")
            bn2 = pp.tile([128, 4], F32, name="bn2", tag="bn2")
            nc.sync.dma_start(bn1[:], bn1_d[:])
            nc.sync.dma_start(bn2[:], bn2_d[:])

            def conv_block(catp, wbd, pwT, bn, xout):
                with tc.tile_pool(name="conv_sb", bufs=1) as csb, \
                     tc.tile_pool(name="conv_y", bufs=2) as cyb, \
                     tc.tile_pool(name="conv_ps", bufs=2, space="PSUM") as cps:
                    cat_sb = [csb.tile([128, CAT_F], BF16,
                                       name=f"cat_{k}", tag=f"cat_{k}")
                              for k in range(4)]
                    w_sb = [csb.tile([128, 9 * 128], BF16,
                                     name=f"wbd_{k}", tag=f"wbd_{k}")
                            for k in range(4)]
                    pw_sb = [csb.tile([128, 256], BF16,
                                      name=f"pwT_{k}", tag=f"pwT_{k}")
                             for k in range(4)]
                    for k in range(4):
                        nc.sync.dma_start(w_sb[k][:], wbd[k])
                        nc.sync.dma_start(pw_sb[k][:], pwT[k])
                        nc.sync.dma_start(cat_sb[k][:], catp[k])
                    for w in range(4):
                        y1w = [cyb.tile([128, 512], BF16,
                                        name=f"y1w_{k}", tag=f"y1w_{k}")
                               for k in range(4)]
                        for k in range(4):
                            ps = cps.tile([128, 512], F32, name="dwps", tag="dwps")
                            for t in range(9):
                                dr, dc = t // 3, t % 3
                                start = OFF + (8 * w + dr) * PW + dc - 1
                                rhs = cat_sb[k][:, start:start + 8 * PW] \
                                    .rearrange("p (r c) -> p r c", r=8, c=PW)[:, :, 0:64]
                                nc.tensor.matmul(
                                    ps[:], w_sb[k][:, 128 * t:128 * (t + 1)], rhs,
                                    start=(t == 0), stop=(t == 8))
                            nc.scalar.activation(y1w[k][:], ps[:], AF.Copy)
                        for m in range(2):
                            pp2 = cps.tile([128, 512], F32, name="pwps", tag="pwps")
                            for k in range(4):
                                nc.tensor.matmul(
                                    pp2[:], pw_sb[k][:, 128 * m:128 * (m + 1)],
                                    y1w[k][:], start=(k == 0), stop=(k == 3))
                            nc.scalar.activation(
                                xout[m][:, 512 * w:512 * (w + 1)], pp2[:],
                                AF.Relu, bias=bn[:, 2 * m + 1:2 * m + 2],
                                scale=bn[:, 2 * m:2 * m + 1])

            conv_block(cat1p, w1bd, pw1T, bn1, x3o)

            # ---- own-half projections: k_own, v^T_own ----
            pp2c = tc.tile_pool(name="persist2", bufs=1)
            p2 = pp2c.__enter__()
            k_own = p2.tile([128, QH], BF16, name="k_own", tag="k_own")
            vto = p2.tile([128, 16 * VT], BF16, name="vto", tag="vto")
            nc.sync.dma_start(vto[:], vinit_d[:])
            k_sb = p2.tile([128, N], BF16, name="k_sb", tag="k_sb")
            q_sb = p2.tile([128, QH], BF16, name="q_sb", tag="q_sb")
            vta = p2.tile([128, 32 * VT], BF16, name="vta", tag="vta")
            ident = p2.tile([128, 128], F32R, name="ident", tag="ident")
            bkq = p2.tile([128, 1], F32, name="bkq", tag="bkq")
            bvg = p2.tile([128, 2], F32, name="bvg", tag="bvg")
            nc.sync.dma_start(ident[:], ident_d[:])
            nc.sync.dma_start(bkq[:], bkq_d[:])
            for ch in range(2):
                nc.sync.dma_start(bvg[:, ch:ch + 1], bvg_d[ch])

            with tc.tile_pool(name="proj_sb", bufs=1) as psb, \
                 tc.tile_pool(name="proj_ps", bufs=2, space="PSUM") as pps:
                wv_sb = [psb.tile([128, 256], BF16, name=f"wv_{c}", tag=f"wv_{c}")
                         for c in range(2)]
                wk_sb = [psb.tile([128, 32], BF16, name=f"wk_{c}", tag=f"wk_{c}")
                         for c in range(2)]
                wq_sb = [psb.tile([128, 32], BF16, name=f"wq_{c}", tag=f"wq_{c}")
                         for c in range(2)]
                for ch in range(2):
                    nc.sync.dma_start(wv_sb[ch][:], wvT[ch])
                    nc.sync.dma_start(wk_sb[ch][:], wkT[ch])
                    nc.sync.dma_start(wq_sb[ch][:], wqT[ch])
                for s in range(4):
                    ps = pps.tile([128, 512], F32, name="kqps", tag="kqps")
                    for ch in range(2):
                        nc.tensor.matmul(ps[0:32, :], wk_sb[ch][:],
                                         x3o[ch][:, 512 * s:512 * (s + 1)],
                                         start=(ch == 0), stop=(ch == 1))
                    nc.scalar.activation(k_own[0:32, 512 * s:512 * (s + 1)],
                                         ps[0:32, :], AF.Identity,
                                         bias=bkq[0:32, 0:1])
                for j in range(16):
                    ps = pps.tile([128, 256], F32, name="vtps", tag="vtps")
                    for ch in range(2):
                        nc.tensor.matmul(ps[:], x3o[ch][:, 128 * j:128 * (j + 1)],
                                         wv_sb[ch][:], start=(ch == 0), stop=(ch == 1))
                    nc.scalar.activation(vto[:, VT * j:VT * j + 256], ps[:], AF.Copy)

                # ship own k / v^T, gather pair halves
                nc.sync.dma_start(kown_d[:], k_own[0:32, :])
                nc.gpsimd.collective_compute(
                    "AllGather", ALU.bypass, replica_groups=PAIRS,
                    ins=[kown_d[:]], outs=[kfull_d[:]])
                nc.sync.dma_start(vown_d[:], vto[:])
                nc.gpsimd.collective_compute(
                    "AllGather", ALU.bypass, replica_groups=PAIRS,
                    ins=[vown_d[:].opt()], outs=[vfull_d[:].opt()])
                nc.sync.dma_start(k_sb[0:32, 0:QH], kfull_d[0:32, :])
                nc.sync.dma_start(k_sb[0:32, QH:N], kfull_d[32:64, :])
                nc.sync.dma_start(vta[:, 0:16 * VT], vfull_d[0])
                nc.sync.dma_start(vta[:, 16 * VT:32 * VT], vfull_d[1])

                # conv2 + q overlap with the collective
                conv_block(cat2p, w2bd, pw2T, bn2, x4)
                for s in range(4):
                    ps = pps.tile([128, 512], F32, name="kqps", tag="kqps")
                    for ch in range(2):
                        nc.tensor.matmul(ps[0:32, :], wq_sb[ch][:],
                                         x4[ch][:, 512 * s:512 * (s + 1)],
                                         start=(ch == 0), stop=(ch == 1))
                    nc.scalar.activation(q_sb[0:32, 512 * s:512 * (s + 1)],
                                         ps[0:32, :], AF.Identity,
                                         bias=bkq[32:64, 0:1])


            # ---- flash attention ----
            x1h = [p2.tile([128, QH], F32, name=f"x1h_{c}", tag=f"x1h_{c}")
                   for c in range(2)]
            out_sb = [p2.tile([128, QH], F32, name=f"osb_{c}", tag=f"osb_{c}")
                      for c in range(2)]
            for ch in range(2):
                nc.sync.dma_start(x1h[ch][:], x1h_d[ch])

            with tc.tile_pool(name="att_sb", bufs=2) as asb, \
                 tc.tile_pool(name="acc_ps", bufs=4, space="PSUM") as accp, \
                 tc.tile_pool(name="e_ps", bufs=2, space="PSUM") as epsp, \
                 tc.tile_pool(name="t_ps", bufs=2, space="PSUM") as tpsp:
                for ib in range(4):
                    acc = [accp.tile([128, VT], F32, name="acc", tag="acc")
                           for _ in range(4)]
                    eps_t = {}
                    for j in range(32):
                        if j == 0:
                            eps_t[0] = epsp.tile([128, 512], F32, name="eps",
                                                 tag="eps")
                            nc.tensor.matmul(eps_t[0][:],
                                             k_sb[0:32, 0:128],
                                             q_sb[0:32, 512 * ib:512 * (ib + 1)],
                                             start=True, stop=True)
                        ex = asb.tile([128, 512], BF16, name="ex", tag="ex")
                        nc.scalar.activation(ex[:], eps_t[j][:], AF.Exp)
                        if j + 1 < 32:
                            eps_t[j + 1] = epsp.tile([128, 512], F32, name="eps",
                                                     tag="eps")
                            nc.tensor.matmul(eps_t[j + 1][:],
                                             k_sb[0:32, 128 * (j + 1):128 * (j + 2)],
                                             q_sb[0:32, 512 * ib:512 * (ib + 1)],
                                             start=True, stop=True)
                        eps_t.pop(j - 1, None)
                        for cq in range(4):
                            nc.tensor.matmul(acc[cq][:],
                                             ex[:, 128 * cq:128 * (cq + 1)],
                                             vta[:, VT * j:VT * (j + 1)],
                                             start=(j == 0), stop=(j == 31))
                    for cq in range(4):
                        ic = 4 * ib + cq
                        rec = asb.tile([128, 1], F32, name="rec", tag="rec")
                        nc.vector.reciprocal(rec[:], acc[cq][:, 256:257])
                        att = asb.tile([128, 256], F32R, name="att", tag="att")
                        nc.scalar.activation(att[:], acc[cq][:, 0:256], AF.Identity,
                                             bias=0.0, scale=rec[:, 0:1])
                        for ch in range(2):
                            tp = tpsp.tile([128, 128], F32, name="tp", tag="tp")
                            nc.tensor.transpose(tp[:].bitcast(F32R),
                                                att[:, 128 * ch:128 * (ch + 1)],
                                                ident[:])
                            nc.vector.scalar_tensor_tensor(
                                out_sb[ch][:, 128 * ic:128 * (ic + 1)], tp[:],
                                bvg[:, ch:ch + 1],
                                x1h[ch][:, 128 * ic:128 * (ic + 1)],
                                ALU.add, ALU.add)
                    for ch in range(2):
                        nc.sync.dma_start(
                            out_d[ch][:, 512 * ib:512 * (ib + 1)],
                            out_sb[ch][:, 512 * ib:512 * (ib + 1)])
            pp2c.__exit__(None, None, None)
    nc.compile()
    return nc


def _prep_shared(inputs):
    f = np.float32
    bf = ml_dtypes.bfloat16

    def bd(w_dw):
        wr = w_dw.reshape(512, 2, 9)
        Wt = np.zeros((4, 128, 9, 128), f)
        m = np.arange(64)
        for k in range(4):
            blk = wr[128 * k:128 * (k + 1)]        # [128, 2, 9]
            for i in range(2):
                for j in range(2):
                    Wt[k, 2 * m + i, :, 2 * m + j] = blk[2 * m + j, i, :]
        return np.ascontiguousarray(Wt.reshape(4, 128, 9 * 128)).astype(bf)

    w1bd = bd(inputs["w1_dw"])
    w2bd = bd(inputs["w2_dw"])

    pw1 = inputs["w1_pw"][:, :, 0, 0]              # [256, 512]
    pw2 = inputs["w2_pw"][:, :, 0, 0]
    pw1T = np.ascontiguousarray(pw1.T.reshape(4, 128, 256)).astype(bf)
    pw2T = np.ascontiguousarray(pw2.T.reshape(4, 128, 256)).astype(bf)

    gamma = float(inputs["gamma"][0])
    wvTg = np.ascontiguousarray(
        (inputs["wv"][:, :, 0, 0].T * gamma).reshape(2, 128, 256).astype(bf))
    wkT = np.ascontiguousarray(
        inputs["wk"][:, :, 0, 0].T.reshape(2, 128, 32)).astype(bf)
    wqT = np.ascontiguousarray(
        inputs["wq"][:, :, 0, 0].T.reshape(2, 128, 32)).astype(bf)

    def bn_fold(g, b_, mean, var, pw, b_dw, b_pw):
        s = g / np.sqrt(var + EPS)
        bc = pw @ b_dw + b_pw
        t = s * (bc - mean) + b_
        o = np.zeros((128, 4), f)
        o[:, 0], o[:, 1] = s[0:128], t[0:128]
        o[:, 2], o[:, 3] = s[128:256], t[128:256]
        return o

    bn1 = bn_fold(inputs["bn1_g"], inputs["bn1_b"], inputs["bn1_m"],
                  inputs["bn1_v"], pw1, inputs["b1_dw"], inputs["b1_pw"])
    bn2 = bn_fold(inputs["bn2_g"], inputs["bn2_b"], inputs["bn2_m"],
                  inputs["bn2_v"], pw2, inputs["b2_dw"], inputs["b2_pw"])

    bkq = np.zeros((128, 1), f)
    bkq[0:32, 0] = inputs["bk"]
    bkq[32:64, 0] = inputs["bq"]
    bvg = np.ascontiguousarray((gamma * inputs["bv"]).reshape(2, 128, 1).astype(f))
    ident = np.ascontiguousarray(np.eye(128, dtype=f))

    vinit = np.zeros((128, 16 * VT), bf)
    for j in range(16):
        vinit[:, VT * j + 256] = 1.0

    return dict(w1bd=w1bd, w2bd=w2bd, pw1T=pw1T, pw2T=pw2T, wvT=wvTg,
                wkT=wkT, wqT=wqT, bn1=bn1, bn2=bn2, bkq=bkq, bvg=bvg,
                ident=ident, vinit=vinit)


def _prep_core(inputs, b, h):
    bf = ml_dtypes.bfloat16
    x1 = inputs["x1"][b]          # [256, 64, 64]
    x2 = inputs["x2"][b]
    sub = x1 - x2
    cat1 = np.concatenate([sub, x1], axis=0).reshape(4, 128, 64, 64)
    cat2 = np.concatenate([sub, x2], axis=0).reshape(4, 128, 64, 64)

    def pad_half(cc):
        buf = np.zeros((4, 128, SLOTS, 66), np.float32)
        if h == 0:
            buf[:, :, 1:34, 1:65] = cc[:, :, 0:33, :]
        else:
            buf[:, :, 0:33, 1:65] = cc[:, :, 31:64, :]
        catp = np.zeros((4, 128, CAT_F), bf)
        catp[:, :, OFF:OFF + SLOTS * PW] = buf.reshape(4, 128, -1)
        return catp

    x1h = np.ascontiguousarray(
        x1.reshape(256, N)[:, QH * h:QH * (h + 1)].reshape(2, 128, QH))
    return dict(cat1p=pad_half(cat1), cat2p=pad_half(cat2), x1h=x1h)


def kernel(**inputs):
    if "nc" not in _CACHE:
        _CACHE["nc"] = _build_nc()
    nc = _CACHE["nc"]

    inputs = {k: np.ascontiguousarray(np.asarray(v)) for k, v in inputs.items()}
    shared = _prep_shared(inputs)
    in_maps = []
    for core in range(8):
        b, h = core // 2, core % 2
        m = dict(shared)
        m.update(_prep_core(inputs, b, h))
        in_maps.append(m)

    res = run_bass_kernel_spmd(nc, in_maps, list(range(8)))
    out = np.empty((4, 256, N), np.float32)
    for core in range(8):
        b, h = core // 2, core % 2
        r = res.results[core]["out"]
        out[b, 0:128, QH * h:QH * (h + 1)] = r[0]
        out[b, 128:256, QH * h:QH * (h + 1)] = r[1]
    return out.reshape(B, C, H, W)



# revision 3
# speedup vs baseline: 1.2103x; 1.2103x over previous
"""CrossAttention kernel for Trainium2, 8 NeuronCores.

Reference pipeline (B=4, C=256, H=W=64, N=4096, d=C//8=32):
  sub = x1 - x2
  x3 = relu(bn1(pw1(dw1([sub, x1]))))      # dw: 3x3 grouped conv (groups=C)
  x4 = relu(bn2(pw2(dw2([sub, x2]))))      # pw: 1x1 512->256
  q = wq@x4 [B,32,N]; k = wk@x3 [B,32,N]; v = wv@x3 [B,256,N]
  attn = softmax(q^T k);  out = gamma * (v @ attn^T) + x1

Sharding: 8 cores = (batch b) x (pixel-half h). Each core computes BOTH
conv paths only for its own pixel half (with a one-row halo), projects
k / v^T / q from its half, then the pair exchanges k and v^T via
AllGather so each core can run flash attention for its 2048 queries over
all 4096 keys.

Device-side choices:
  - dw conv on the PE as 9 accumulating block-diagonal [128x128] matmuls
    over a zero-padded 66-col image layout (8-row windows = 512 output
    columns via a strided rhs AP that skips the pad columns).
  - energy is computed transposed, E^T[j, i]; the softmax denominator
    comes for free from an appended ones-column in v^T in the second
    (accumulating) matmul.
  - matmul datapath in bf16 (fp32 PSUM accumulation); normalize /
    transpose / residual-add in fp32.
  - gamma folded into wv/bv on the host; bn+biases folded into per-
    channel scale/shift applied by ScalarE during PSUM eviction.
"""

import numpy as np
import ml_dtypes

import concourse.bass as bass
import concourse.mybir as mybir
import concourse.tile as tile
from concourse import bacc
from concourse.bass_utils import run_bass_kernel_spmd

F32 = mybir.dt.float32
BF16 = mybir.dt.bfloat16
AF = mybir.ActivationFunctionType
ALU = mybir.AluOpType

B, C, H, W = 4, 256, 64, 64
N = H * W            # 4096 pixels
QH = N // 2          # pixels per core (queries/own keys)
EPS = 1e-5
PW = 66              # padded row width
OFF = 2              # leading pad elements in padded tiles
SLOTS = 34           # 32 data rows + halo/zero rows
CAT_F = OFF + SLOTS * PW + OFF   # 2248
VT = 258             # v^T row: 256 channels + ones + pad
PAIRS = [[0, 1], [2, 3], [4, 5], [6, 7]]

_CACHE = {}


def _build_nc():
    nc = bacc.Bacc("TRN2", target_bir_lowering=False, debug=False, num_devices=8)

    cat1p = nc.dram_tensor("cat1p", [4, 128, CAT_F], BF16, kind="ExternalInput")
    cat2p = nc.dram_tensor("cat2p", [4, 128, CAT_F], BF16, kind="ExternalInput")
    x1h_d = nc.dram_tensor("x1h", [2, 128, QH], F32, kind="ExternalInput")
    w1bd = nc.dram_tensor("w1bd", [4, 128, 9 * 128], BF16, kind="ExternalInput")
    w2bd = nc.dram_tensor("w2bd", [4, 128, 9 * 128], BF16, kind="ExternalInput")
    pw1T = nc.dram_tensor("pw1T", [4, 128, 256], BF16, kind="ExternalInput")
    pw2T = nc.dram_tensor("pw2T", [4, 128, 256], BF16, kind="ExternalInput")
    wvT = nc.dram_tensor("wvT", [2, 128, 256], BF16, kind="ExternalInput")
    wkT = nc.dram_tensor("wkT", [2, 128, 32], BF16, kind="ExternalInput")
    wqT = nc.dram_tensor("wqT", [2, 128, 32], BF16, kind="ExternalInput")
    bn1_d = nc.dram_tensor("bn1", [128, 4], F32, kind="ExternalInput")
    bn2_d = nc.dram_tensor("bn2", [128, 4], F32, kind="ExternalInput")
    bkq_d = nc.dram_tensor("bkq", [128, 1], F32, kind="ExternalInput")
    bvg_d = nc.dram_tensor("bvg", [2, 128, 1], F32, kind="ExternalInput")
    vinit_d = nc.dram_tensor("vinit", [128, 16 * VT], BF16, kind="ExternalInput")
    out_d = nc.dram_tensor("out", [2, 128, QH], F32, kind="ExternalOutput")

    # collective bounce buffers
    kown_d = nc.dram_tensor("kown_b", [32, QH], BF16)
    kfull_d = nc.dram_tensor("kfull_b", [64, QH], BF16)
    vown_d = nc.dram_tensor("vown_b", [128, 16 * VT], BF16)
    vfull_d = nc.dram_tensor("vfull_b", [2, 128, 16 * VT], BF16)

    with tile.TileContext(nc) as tc:
        with tc.tile_pool(name="persist", bufs=1) as pp:
            x3o = [pp.tile([128, QH], BF16, name=f"x3o_{m}", tag=f"x3o_{m}")
                   for m in range(2)]
            x4 = [pp.tile([128, QH], BF16, name=f"x4_{m}", tag=f"x4_{m}")
                  for m in range(2)]
            bn1 = pp.tile([128, 4], F32, name="bn1", tag="bn1")
            bn2 = pp.tile([128, 4], F32, name="bn2", tag="bn2")
            nc.sync.dma_start(bn1[:], bn1_d[:])
            nc.sync.dma_start(bn2[:], bn2_d[:])

            def conv_block(catp, wbd, pwT, bn, xout):
                with tc.tile_pool(name="conv_sb", bufs=1) as csb, \
                     tc.tile_pool(name="conv_y", bufs=2) as cyb, \
                     tc.tile_pool(name="conv_ps", bufs=2, space="PSUM") as cps:
                    cat_sb = [csb.tile([128, CAT_F], BF16,
                                       name=f"cat_{k}", tag=f"cat_{k}")
                              for k in range(4)]
                    w_sb = [csb.tile([128, 9 * 128], BF16,
                                     name=f"wbd_{k}", tag=f"wbd_{k}")
                            for k in range(4)]
                    pw_sb = [csb.tile([128, 256], BF16,
                                      name=f"pwT_{k}", tag=f"pwT_{k}")
                             for k in range(4)]
                    for k in range(4):
                        nc.sync.dma_start(w_sb[k][:], wbd[k])
                        nc.sync.dma_start(pw_sb[k][:], pwT[k])
                        nc.sync.dma_start(cat_sb[k][:], catp[k])
                    for w in range(4):
                        y1w = [cyb.tile([128, 512], BF16,
                                        name=f"y1w_{k}", tag=f"y1w_{k}")
                               for k in range(4)]
                        for k in range(4):
                            ps = cps.tile([128, 512], F32, name="dwps", tag="dwps")
                            for t in range(9):
                                dr, dc = t // 3, t % 3
                                start = OFF + (8 * w + dr) * PW + dc - 1
                                rhs = cat_sb[k][:, start:start + 8 * PW] \
                                    .rearrange("p (r c) -> p r c", r=8, c=PW)[:, :, 0:64]
                                nc.tensor.matmul(
                                    ps[:], w_sb[k][:, 128 * t:128 * (t + 1)], rhs,
                                    start=(t == 0), stop=(t == 8))
                            nc.scalar.activation(y1w[k][:], ps[:], AF.Copy)
                        for m in range(2):
                            pp2 = cps.tile([128, 512], F32, name="pwps", tag="pwps")
                            for k in range(4):
                                nc.tensor.matmul(
                                    pp2[:], pw_sb[k][:, 128 * m:128 * (m + 1)],
                                    y1w[k][:], start=(k == 0), stop=(k == 3))
                            nc.scalar.activation(
                                xout[m][:, 512 * w:512 * (w + 1)], pp2[:],
                                AF.Relu, bias=bn[:, 2 * m + 1:2 * m + 2],
                                scale=bn[:, 2 * m:2 * m + 1])

            conv_block(cat1p, w1bd, pw1T, bn1, x3o)

            # ---- own-half projections: k_own, v^T_own ----
            pp2c = tc.tile_pool(name="persist2", bufs=1)
            p2 = pp2c.__enter__()
            k_own = p2.tile([128, QH], BF16, name="k_own", tag="k_own")
            vto = p2.tile([128, 16 * VT], BF16, name="vto", tag="vto")
            nc.sync.dma_start(vto[:], vinit_d[:])
            k_sb = p2.tile([128, N], BF16, name="k_sb", tag="k_sb")
            q_sb = p2.tile([128, QH], BF16, name="q_sb", tag="q_sb")
            vta = p2.tile([128, 32 * VT], BF16, name="vta", tag="vta")
            bkq = p2.tile([128, 1], F32, name="bkq", tag="bkq")
            bvg = p2.tile([128, 2], F32, name="bvg", tag="bvg")
            nc.sync.dma_start(bkq[:], bkq_d[:])
            for ch in range(2):
                nc.sync.dma_start(bvg[:, ch:ch + 1], bvg_d[ch])

            with tc.tile_pool(name="proj_sb", bufs=1) as psb, \
                 tc.tile_pool(name="proj_ps", bufs=2, space="PSUM") as pps:
                wv_sb = [psb.tile([128, 256], BF16, name=f"wv_{c}", tag=f"wv_{c}")
                         for c in range(2)]
                wk_sb = [psb.tile([128, 32], BF16, name=f"wk_{c}", tag=f"wk_{c}")
                         for c in range(2)]
                wq_sb = [psb.tile([128, 32], BF16, name=f"wq_{c}", tag=f"wq_{c}")
                         for c in range(2)]
                for ch in range(2):
                    nc.sync.dma_start(wv_sb[ch][:], wvT[ch])
                    nc.sync.dma_start(wk_sb[ch][:], wkT[ch])
                    nc.sync.dma_start(wq_sb[ch][:], wqT[ch])
                for s in range(4):
                    ps = pps.tile([128, 512], F32, name="kqps", tag="kqps")
                    for ch in range(2):
                        nc.tensor.matmul(ps[0:32, :], wk_sb[ch][:],
                                         x3o[ch][:, 512 * s:512 * (s + 1)],
                                         start=(ch == 0), stop=(ch == 1))
                    nc.scalar.activation(k_own[0:32, 512 * s:512 * (s + 1)],
                                         ps[0:32, :], AF.Identity,
                                         bias=bkq[0:32, 0:1])
                for j in range(16):
                    ps = pps.tile([128, 256], F32, name="vtps", tag="vtps")
                    for ch in range(2):
                        nc.tensor.matmul(ps[:], x3o[ch][:, 128 * j:128 * (j + 1)],
                                         wv_sb[ch][:], start=(ch == 0), stop=(ch == 1))
                    nc.scalar.activation(vto[:, VT * j:VT * j + 256], ps[:], AF.Copy)

                # ship own k / v^T, gather pair halves
                nc.sync.dma_start(kown_d[:], k_own[0:32, :])
                nc.gpsimd.collective_compute(
                    "AllGather", ALU.bypass, replica_groups=PAIRS,
                    ins=[kown_d[:]], outs=[kfull_d[:]])
                nc.sync.dma_start(vown_d[:], vto[:])
                nc.gpsimd.collective_compute(
                    "AllGather", ALU.bypass, replica_groups=PAIRS,
                    ins=[vown_d[:].opt()], outs=[vfull_d[:].opt()])
                nc.sync.dma_start(k_sb[0:32, 0:QH], kfull_d[0:32, :])
                nc.sync.dma_start(k_sb[0:32, QH:N], kfull_d[32:64, :])
                nc.sync.dma_start(vta[:, 0:16 * VT], vfull_d[0])
                nc.sync.dma_start(vta[:, 16 * VT:32 * VT], vfull_d[1])

                # conv2 + q overlap with the collective
                conv_block(cat2p, w2bd, pw2T, bn2, x4)
                for s in range(4):
                    ps = pps.tile([128, 512], F32, name="kqps", tag="kqps")
                    for ch in range(2):
                        nc.tensor.matmul(ps[0:32, :], wq_sb[ch][:],
                                         x4[ch][:, 512 * s:512 * (s + 1)],
                                         start=(ch == 0), stop=(ch == 1))
                    nc.scalar.activation(q_sb[0:32, 512 * s:512 * (s + 1)],
                                         ps[0:32, :], AF.Identity,
                                         bias=bkq[32:64, 0:1])


            # ---- flash attention (output computed pre-transposed) ----
            # acc_c[c, i] = sum_j v^T[j, c] * ex[j, i]  via lhsT=vta block,
            # rhs=ex: 512-wide matmuls, output lands as [channel, query] so
            # no PE transposes are needed. The softmax denominator comes
            # from an all-ones lhsT matmul, which also broadcasts it across
            # all 128 partitions for the DVE normalize.
            x1h = [p2.tile([128, QH], F32, name=f"x1h_{c}", tag=f"x1h_{c}")
                   for c in range(2)]
            out_sb = [p2.tile([128, QH], F32, name=f"osb_{c}", tag=f"osb_{c}")
                      for c in range(2)]
            ones_sb = p2.tile([128, 128], BF16, name="ones", tag="ones")
            nc.gpsimd.memset(ones_sb[:], 1.0)
            for ch in range(2):
                nc.sync.dma_start(x1h[ch][:], x1h_d[ch])

            with tc.tile_pool(name="att_sb", bufs=2) as asb, \
                 tc.tile_pool(name="acc_ps", bufs=2, space="PSUM") as accp, \
                 tc.tile_pool(name="e_ps", bufs=2, space="PSUM") as epsp:
                for ib in range(4):
                    acc = [accp.tile([128, 512], F32, name=f"acc{c}",
                                     tag=f"acc{c}") for c in range(2)]
                    dps = accp.tile([128, 512], F32, name="dps", tag="dps")
                    eps_t = {}
                    for j in range(32):
                        if j == 0:
                            eps_t[0] = epsp.tile([128, 512], F32, name="eps",
                                                 tag="eps")
                            nc.tensor.matmul(eps_t[0][:],
                                             k_sb[0:32, 0:128],
                                             q_sb[0:32, 512 * ib:512 * (ib + 1)],
                                             start=True, stop=True)
                        ex = asb.tile([128, 512], BF16, name="ex", tag="ex")
                        nc.scalar.activation(ex[:], eps_t[j][:], AF.Exp)
                        if j + 1 < 32:
                            eps_t[j + 1] = epsp.tile([128, 512], F32, name="eps",
                                                     tag="eps")
                            nc.tensor.matmul(eps_t[j + 1][:],
                                             k_sb[0:32, 128 * (j + 1):128 * (j + 2)],
                                             q_sb[0:32, 512 * ib:512 * (ib + 1)],
                                             start=True, stop=True)
                        eps_t.pop(j - 1, None)
                        nc.tensor.matmul(acc[0][:],
                                         vta[:, VT * j:VT * j + 128], ex[:],
                                         start=(j == 0), stop=(j == 31))
                        nc.tensor.matmul(acc[1][:],
                                         vta[:, VT * j + 128:VT * j + 256], ex[:],
                                         start=(j == 0), stop=(j == 31))
                        nc.tensor.matmul(dps[:], ones_sb[:], ex[:],
                                         start=(j == 0), stop=(j == 31))
                    rec = asb.tile([128, 512], F32, name="rec", tag="rec")
                    nc.vector.reciprocal(rec[:], dps[:])
                    for ch in range(2):
                        tmp = asb.tile([128, 512], F32, name="tmp", tag="tmp")
                        nc.vector.scalar_tensor_tensor(
                            tmp[:], acc[ch][:], 1.0, rec[:],
                            ALU.mult, ALU.mult)
                        nc.vector.scalar_tensor_tensor(
                            out_sb[ch][:, 512 * ib:512 * (ib + 1)], tmp[:],
                            bvg[:, ch:ch + 1],
                            x1h[ch][:, 512 * ib:512 * (ib + 1)],
                            ALU.add, ALU.add)
                        nc.sync.dma_start(
                            out_d[ch][:, 512 * ib:512 * (ib + 1)],
                            out_sb[ch][:, 512 * ib:512 * (ib + 1)])
            pp2c.__exit__(None, None, None)
    nc.compile()
    return nc


def _prep_shared(inputs):
    f = np.float32
    bf = ml_dtypes.bfloat16

    def bd(w_dw):
        wr = w_dw.reshape(512, 2, 9)
        Wt = np.zeros((4, 128, 9, 128), f)
        m = np.arange(64)
        for k in range(4):
            blk = wr[128 * k:128 * (k + 1)]        # [128, 2, 9]
            for i in range(2):
                for j in range(2):
                    Wt[k, 2 * m + i, :, 2 * m + j] = blk[2 * m + j, i, :]
        return np.ascontiguousarray(Wt.reshape(4, 128, 9 * 128)).astype(bf)

    w1bd = bd(inputs["w1_dw"])
    w2bd = bd(inputs["w2_dw"])

    pw1 = inputs["w1_pw"][:, :, 0, 0]              # [256, 512]
    pw2 = inputs["w2_pw"][:, :, 0, 0]
    pw1T = np.ascontiguousarray(pw1.T.reshape(4, 128, 256)).astype(bf)
    pw2T = np.ascontiguousarray(pw2.T.reshape(4, 128, 256)).astype(bf)

    gamma = float(inputs["gamma"][0])
    wvTg = np.ascontiguousarray(
        (inputs["wv"][:, :, 0, 0].T * gamma).reshape(2, 128, 256).astype(bf))
    wkT = np.ascontiguousarray(
        inputs["wk"][:, :, 0, 0].T.reshape(2, 128, 32)).astype(bf)
    wqT = np.ascontiguousarray(
        inputs["wq"][:, :, 0, 0].T.reshape(2, 128, 32)).astype(bf)

    def bn_fold(g, b_, mean, var, pw, b_dw, b_pw):
        s = g / np.sqrt(var + EPS)
        bc = pw @ b_dw + b_pw
        t = s * (bc - mean) + b_
        o = np.zeros((128, 4), f)
        o[:, 0], o[:, 1] = s[0:128], t[0:128]
        o[:, 2], o[:, 3] = s[128:256], t[128:256]
        return o

    bn1 = bn_fold(inputs["bn1_g"], inputs["bn1_b"], inputs["bn1_m"],
                  inputs["bn1_v"], pw1, inputs["b1_dw"], inputs["b1_pw"])
    bn2 = bn_fold(inputs["bn2_g"], inputs["bn2_b"], inputs["bn2_m"],
                  inputs["bn2_v"], pw2, inputs["b2_dw"], inputs["b2_pw"])

    bkq = np.zeros((128, 1), f)
    bkq[0:32, 0] = inputs["bk"]
    bkq[32:64, 0] = inputs["bq"]
    bvg = np.ascontiguousarray((gamma * inputs["bv"]).reshape(2, 128, 1).astype(f))

    vinit = np.zeros((128, 16 * VT), bf)
    for j in range(16):
        vinit[:, VT * j + 256] = 1.0

    return dict(w1bd=w1bd, w2bd=w2bd, pw1T=pw1T, pw2T=pw2T, wvT=wvTg,
                wkT=wkT, wqT=wqT, bn1=bn1, bn2=bn2, bkq=bkq, bvg=bvg,
                vinit=vinit)


def _prep_core(inputs, b, h):
    bf = ml_dtypes.bfloat16
    x1 = inputs["x1"][b]          # [256, 64, 64]
    x2 = inputs["x2"][b]
    sub = x1 - x2
    cat1 = np.concatenate([sub, x1], axis=0).reshape(4, 128, 64, 64)
    cat2 = np.concatenate([sub, x2], axis=0).reshape(4, 128, 64, 64)

    def pad_half(cc):
        buf = np.zeros((4, 128, SLOTS, 66), np.float32)
        if h == 0:
            buf[:, :, 1:34, 1:65] = cc[:, :, 0:33, :]
        else:
            buf[:, :, 0:33, 1:65] = cc[:, :, 31:64, :]
        catp = np.zeros((4, 128, CAT_F), bf)
        catp[:, :, OFF:OFF + SLOTS * PW] = buf.reshape(4, 128, -1)
        return catp

    x1h = np.ascontiguousarray(
        x1.reshape(256, N)[:, QH * h:QH * (h + 1)].reshape(2, 128, QH))
    return dict(cat1p=pad_half(cat1), cat2p=pad_half(cat2), x1h=x1h)


def kernel(**inputs):
    if "nc" not in _CACHE:
        _CACHE["nc"] = _build_nc()
    nc = _CACHE["nc"]

    inputs = {k: np.ascontiguousarray(np.asarray(v)) for k, v in inputs.items()}
    shared = _prep_shared(inputs)
    in_maps = []
    for core in range(8):
        b, h = core // 2, core % 2
        m = dict(shared)
        m.update(_prep_core(inputs, b, h))
        in_maps.append(m)

    res = run_bass_kernel_spmd(nc, in_maps, list(range(8)))
    out = np.empty((4, 256, N), np.float32)
    for core in range(8):
        b, h = core // 2, core % 2
        r = res.results[core]["out"]
        out[b, 0:128, QH * h:QH * (h + 1)] = r[0]
        out[b, 128:256, QH * h:QH * (h + 1)] = r[1]
    return out.reshape(B, C, H, W)



# revision 7
# speedup vs baseline: 1.2216x; 1.0094x over previous
"""CrossAttention kernel for Trainium2, 8 NeuronCores.

Reference pipeline (B=4, C=256, H=W=64, N=4096, d=C//8=32):
  sub = x1 - x2
  x3 = relu(bn1(pw1(dw1([sub, x1]))))      # dw: 3x3 grouped conv (groups=C)
  x4 = relu(bn2(pw2(dw2([sub, x2]))))      # pw: 1x1 512->256
  q = wq@x4 [B,32,N]; k = wk@x3 [B,32,N]; v = wv@x3 [B,256,N]
  attn = softmax(q^T k);  out = gamma * (v @ attn^T) + x1

Sharding: 8 cores = (batch b) x (pixel-half h). Each core computes BOTH
conv paths only for its own pixel half (with a one-row halo), projects
k / v^T / q from its half, then the pair exchanges k and v^T via
AllGather so each core can run flash attention for its 2048 queries over
all 4096 keys.

Device-side choices:
  - dw conv on the PE as 9 accumulating block-diagonal [128x128] matmuls
    over a zero-padded 66-col image layout (8-row windows = 512 output
    columns via a strided rhs AP that skips the pad columns).
  - energy is computed transposed, E^T[j, i]; the softmax denominator
    comes for free from an appended ones-column in v^T in the second
    (accumulating) matmul.
  - matmul datapath in bf16 (fp32 PSUM accumulation); normalize /
    transpose / residual-add in fp32.
  - gamma folded into wv/bv on the host; bn+biases folded into per-
    channel scale/shift applied by ScalarE during PSUM eviction.
"""

import numpy as np
import ml_dtypes

import concourse.bass as bass
import concourse.mybir as mybir
import concourse.tile as tile
from concourse import bacc
from concourse.bass_utils import run_bass_kernel_spmd

F32 = mybir.dt.float32
BF16 = mybir.dt.bfloat16
AF = mybir.ActivationFunctionType
ALU = mybir.AluOpType

B, C, H, W = 4, 256, 64, 64
N = H * W            # 4096 pixels
QH = N // 2          # pixels per core (queries/own keys)
EPS = 1e-5
PW = 66              # padded row width
OFF = 2              # leading pad elements in padded tiles
SLOTS = 34           # 32 data rows + halo/zero rows
CAT_F = OFF + SLOTS * PW + OFF   # 2248
VT = 258             # v^T row: 256 channels + ones + pad
PAIRS = [[0, 1], [2, 3], [4, 5], [6, 7]]

_CACHE = {}


def _build_nc():
    nc = bacc.Bacc("TRN2", target_bir_lowering=False, debug=False, num_devices=8)

    cat1p = nc.dram_tensor("cat1p", [4, 128, CAT_F], BF16, kind="ExternalInput")
    cat2p = nc.dram_tensor("cat2p", [4, 128, CAT_F], BF16, kind="ExternalInput")
    x1h_d = nc.dram_tensor("x1h", [2, 128, QH], F32, kind="ExternalInput")
    w1bd = nc.dram_tensor("w1bd", [4, 128, 9 * 128], BF16, kind="ExternalInput")
    w2bd = nc.dram_tensor("w2bd", [4, 128, 9 * 128], BF16, kind="ExternalInput")
    pw1T = nc.dram_tensor("pw1T", [4, 128, 256], BF16, kind="ExternalInput")
    pw2T = nc.dram_tensor("pw2T", [4, 128, 256], BF16, kind="ExternalInput")
    wvT = nc.dram_tensor("wvT", [2, 128, 256], BF16, kind="ExternalInput")
    wkT = nc.dram_tensor("wkT", [2, 128, 32], BF16, kind="ExternalInput")
    wqT = nc.dram_tensor("wqT", [2, 128, 32], BF16, kind="ExternalInput")
    bn1_d = nc.dram_tensor("bn1", [128, 4], F32, kind="ExternalInput")
    bn2_d = nc.dram_tensor("bn2", [128, 4], F32, kind="ExternalInput")
    bkq_d = nc.dram_tensor("bkq", [128, 1], F32, kind="ExternalInput")
    bvg_d = nc.dram_tensor("bvg", [2, 128, 1], F32, kind="ExternalInput")
    vinit_d = nc.dram_tensor("vinit", [128, 16 * VT], BF16, kind="ExternalInput")
    out_d = nc.dram_tensor("out", [2, 128, QH], F32, kind="ExternalOutput")

    # collective bounce buffers: one flat gather of v^T (128x4128) + k (32x2048)
    KVN = 128 * 16 * VT + 32 * QH   # 593920
    kvown_d = nc.dram_tensor("kvown_b", [KVN], BF16)
    kvfull_d = nc.dram_tensor("kvfull_b", [2 * KVN], BF16)

    with tile.TileContext(nc) as tc:
        with tc.tile_pool(name="persist", bufs=1) as pp:
            x3o = [pp.tile([128, QH], BF16, name=f"x3o_{m}", tag=f"x3o_{m}")
                   for m in range(2)]
            x4 = [pp.tile([128, QH], BF16, name=f"x4_{m}", tag=f"x4_{m}")
                  for m in range(2)]
            bn1 = pp.tile([128, 4], F32, name="bn1", tag="bn1")
            bn2 = pp.tile([128, 4], F32, name="bn2", tag="bn2")
            nc.sync.dma_start(bn1[:], bn1_d[:])
            nc.sync.dma_start(bn2[:], bn2_d[:])

            def conv_block(catp, wbd, pwT, bn, xout):
                with tc.tile_pool(name="conv_sb", bufs=1) as csb, \
                     tc.tile_pool(name="conv_y", bufs=2) as cyb, \
                     tc.tile_pool(name="conv_ps", bufs=2, space="PSUM") as cps:
                    cat_sb = [csb.tile([128, CAT_F], BF16,
                                       name=f"cat_{k}", tag=f"cat_{k}")
                              for k in range(4)]
                    w_sb = [csb.tile([128, 9 * 128], BF16,
                                     name=f"wbd_{k}", tag=f"wbd_{k}")
                            for k in range(4)]
                    pw_sb = [csb.tile([128, 256], BF16,
                                      name=f"pwT_{k}", tag=f"pwT_{k}")
                             for k in range(4)]
                    for k in range(4):
                        nc.sync.dma_start(w_sb[k][:], wbd[k])
                        nc.sync.dma_start(pw_sb[k][:], pwT[k])
                        nc.sync.dma_start(cat_sb[k][:], catp[k])
                    for w in range(4):
                        y1w = [cyb.tile([128, 512], BF16,
                                        name=f"y1w_{k}", tag=f"y1w_{k}")
                               for k in range(4)]
                        for k in range(4):
                            ps = cps.tile([128, 512], F32, name="dwps", tag="dwps")
                            for t in range(9):
                                dr, dc = t // 3, t % 3
                                start = OFF + (8 * w + dr) * PW + dc - 1
                                rhs = cat_sb[k][:, start:start + 8 * PW] \
                                    .rearrange("p (r c) -> p r c", r=8, c=PW)[:, :, 0:64]
                                nc.tensor.matmul(
                                    ps[:], w_sb[k][:, 128 * t:128 * (t + 1)], rhs,
                                    start=(t == 0), stop=(t == 8))
                            nc.scalar.activation(y1w[k][:], ps[:], AF.Copy)
                        for m in range(2):
                            pp2 = cps.tile([128, 512], F32, name="pwps", tag="pwps")
                            for k in range(4):
                                nc.tensor.matmul(
                                    pp2[:], pw_sb[k][:, 128 * m:128 * (m + 1)],
                                    y1w[k][:], start=(k == 0), stop=(k == 3))
                            nc.scalar.activation(
                                xout[m][:, 512 * w:512 * (w + 1)], pp2[:],
                                AF.Relu, bias=bn[:, 2 * m + 1:2 * m + 2],
                                scale=bn[:, 2 * m:2 * m + 1])

            conv_block(cat1p, w1bd, pw1T, bn1, x3o)

            # ---- own-half projections: k_own, v^T_own ----
            pp2c = tc.tile_pool(name="persist2", bufs=1)
            p2 = pp2c.__enter__()
            k_own = p2.tile([128, QH], BF16, name="k_own", tag="k_own")
            vto = p2.tile([128, 16 * VT], BF16, name="vto", tag="vto")
            nc.sync.dma_start(vto[:], vinit_d[:])
            k_sb = p2.tile([128, N], BF16, name="k_sb", tag="k_sb")
            q_sb = p2.tile([128, QH], BF16, name="q_sb", tag="q_sb")
            # zero rows 32:128 so energy matmuls can use full 128-row lhsT
            # (avoids the PE small-tile row-group slowdown)
            nc.gpsimd.memset(k_sb[:], 0.0)
            nc.gpsimd.memset(q_sb[:], 0.0)
            vta = p2.tile([128, 32 * VT], BF16, name="vta", tag="vta")
            bkq = p2.tile([128, 1], F32, name="bkq", tag="bkq")
            bvg = p2.tile([128, 2], F32, name="bvg", tag="bvg")
            nc.sync.dma_start(bkq[:], bkq_d[:])
            for ch in range(2):
                nc.sync.dma_start(bvg[:, ch:ch + 1], bvg_d[ch])

            with tc.tile_pool(name="proj_sb", bufs=1) as psb, \
                 tc.tile_pool(name="proj_ps", bufs=2, space="PSUM") as pps:
                wv_sb = [psb.tile([128, 256], BF16, name=f"wv_{c}", tag=f"wv_{c}")
                         for c in range(2)]
                wk_sb = [psb.tile([128, 32], BF16, name=f"wk_{c}", tag=f"wk_{c}")
                         for c in range(2)]
                wq_sb = [psb.tile([128, 32], BF16, name=f"wq_{c}", tag=f"wq_{c}")
                         for c in range(2)]
                for ch in range(2):
                    nc.sync.dma_start(wv_sb[ch][:], wvT[ch])
                    nc.sync.dma_start(wk_sb[ch][:], wkT[ch])
                    nc.sync.dma_start(wq_sb[ch][:], wqT[ch])
                for s in range(4):
                    ps = pps.tile([128, 512], F32, name="kqps", tag="kqps")
                    for ch in range(2):
                        nc.tensor.matmul(ps[0:32, :], wk_sb[ch][:],
                                         x3o[ch][:, 512 * s:512 * (s + 1)],
                                         start=(ch == 0), stop=(ch == 1))
                    nc.scalar.activation(k_own[0:32, 512 * s:512 * (s + 1)],
                                         ps[0:32, :], AF.Identity,
                                         bias=bkq[0:32, 0:1])
                for j in range(16):
                    ps = pps.tile([128, 256], F32, name="vtps", tag="vtps")
                    for ch in range(2):
                        nc.tensor.matmul(ps[:], x3o[ch][:, 128 * j:128 * (j + 1)],
                                         wv_sb[ch][:], start=(ch == 0), stop=(ch == 1))
                    nc.scalar.activation(vto[:, VT * j:VT * j + 256], ps[:], AF.Copy)

                # ship own k / v^T, single AllGather for the pair
                VSZ = 128 * 16 * VT
                nc.sync.dma_start(
                    kvown_d[0:VSZ].rearrange("(p f) -> p f", p=128), vto[:])
                nc.sync.dma_start(
                    kvown_d[VSZ:KVN].rearrange("(p f) -> p f", p=32),
                    k_own[0:32, :])
                nc.gpsimd.collective_compute(
                    "AllGather", ALU.bypass, replica_groups=PAIRS,
                    ins=[kvown_d[:].opt()], outs=[kvfull_d[:].opt()])
                for m in range(2):
                    o = m * KVN
                    nc.sync.dma_start(
                        vta[:, 16 * VT * m:16 * VT * (m + 1)],
                        kvfull_d[o:o + VSZ].rearrange("(p f) -> p f", p=128))
                    nc.sync.dma_start(
                        k_sb[0:32, QH * m:QH * (m + 1)],
                        kvfull_d[o + VSZ:o + KVN].rearrange("(p f) -> p f",
                                                            p=32))

                # conv2 + q overlap with the collective
                conv_block(cat2p, w2bd, pw2T, bn2, x4)
                for s in range(4):
                    ps = pps.tile([128, 512], F32, name="kqps", tag="kqps")
                    for ch in range(2):
                        nc.tensor.matmul(ps[0:32, :], wq_sb[ch][:],
                                         x4[ch][:, 512 * s:512 * (s + 1)],
                                         start=(ch == 0), stop=(ch == 1))
                    nc.scalar.activation(q_sb[0:32, 512 * s:512 * (s + 1)],
                                         ps[0:32, :], AF.Identity,
                                         bias=bkq[32:64, 0:1])


            # ---- flash attention (output computed pre-transposed) ----
            # acc_c[c, i] = sum_j v^T[j, c] * ex[j, i]  via lhsT=vta block,
            # rhs=ex: 512-wide matmuls, output lands as [channel, query] so
            # no PE transposes are needed. The softmax denominator comes
            # from an all-ones lhsT matmul, which also broadcasts it across
            # all 128 partitions for the DVE normalize.
            x1h = [p2.tile([128, QH], F32, name=f"x1h_{c}", tag=f"x1h_{c}")
                   for c in range(2)]
            out_sb = [p2.tile([128, QH], F32, name=f"osb_{c}", tag=f"osb_{c}")
                      for c in range(2)]
            ones_sb = p2.tile([128, 128], BF16, name="ones", tag="ones")
            nc.gpsimd.memset(ones_sb[:], 1.0)
            for ch in range(2):
                nc.sync.dma_start(x1h[ch][:], x1h_d[ch])

            with tc.tile_pool(name="att_sb", bufs=2) as asb, \
                 tc.tile_pool(name="acc_ps", bufs=2, space="PSUM") as accp, \
                 tc.tile_pool(name="e_ps", bufs=2, space="PSUM") as epsp:
                for ib in range(4):
                    acc = [accp.tile([128, 512], F32, name=f"acc{c}",
                                     tag=f"acc{c}") for c in range(2)]
                    dps = accp.tile([128, 512], F32, name="dps", tag="dps")
                    eps_t = {}
                    for j in range(32):
                        if j == 0:
                            eps_t[0] = epsp.tile([128, 512], F32, name="eps",
                                                 tag="eps")
                            nc.tensor.matmul(eps_t[0][:],
                                             k_sb[:, 0:128],
                                             q_sb[:, 512 * ib:512 * (ib + 1)],
                                             start=True, stop=True)
                        ex = asb.tile([128, 512], BF16, name="ex", tag="ex")
                        nc.scalar.activation(ex[:], eps_t[j][:], AF.Exp)
                        if j + 1 < 32:
                            eps_t[j + 1] = epsp.tile([128, 512], F32, name="eps",
                                                     tag="eps")
                            nc.tensor.matmul(eps_t[j + 1][:],
                                             k_sb[:, 128 * (j + 1):128 * (j + 2)],
                                             q_sb[:, 512 * ib:512 * (ib + 1)],
                                             start=True, stop=True)
                        eps_t.pop(j - 1, None)
                        nc.tensor.matmul(acc[0][:],
                                         vta[:, VT * j:VT * j + 128], ex[:],
                                         start=(j == 0), stop=(j == 31))
                        nc.tensor.matmul(acc[1][:],
                                         vta[:, VT * j + 128:VT * j + 256], ex[:],
                                         start=(j == 0), stop=(j == 31))
                        nc.tensor.matmul(dps[:], ones_sb[:], ex[:],
                                         start=(j == 0), stop=(j == 31))
                    rec = asb.tile([128, 512], F32, name="rec", tag="rec")
                    nc.vector.reciprocal(rec[:], dps[:])
                    for ch in range(2):
                        tmp = asb.tile([128, 512], F32, name="tmp", tag="tmp")
                        nc.vector.scalar_tensor_tensor(
                            tmp[:], acc[ch][:], 1.0, rec[:],
                            ALU.mult, ALU.mult)
                        nc.vector.scalar_tensor_tensor(
                            out_sb[ch][:, 512 * ib:512 * (ib + 1)], tmp[:],
                            bvg[:, ch:ch + 1],
                            x1h[ch][:, 512 * ib:512 * (ib + 1)],
                            ALU.add, ALU.add)
                        nc.sync.dma_start(
                            out_d[ch][:, 512 * ib:512 * (ib + 1)],
                            out_sb[ch][:, 512 * ib:512 * (ib + 1)])
            pp2c.__exit__(None, None, None)
    nc.compile()
    return nc


def _prep_shared(inputs):
    f = np.float32
    bf = ml_dtypes.bfloat16

    def bd(w_dw):
        wr = w_dw.reshape(512, 2, 9)
        Wt = np.zeros((4, 128, 9, 128), f)
        m = np.arange(64)
        for k in range(4):
            blk = wr[128 * k:128 * (k + 1)]        # [128, 2, 9]
            for i in range(2):
                for j in range(2):
                    Wt[k, 2 * m + i, :, 2 * m + j] = blk[2 * m + j, i, :]
        return np.ascontiguousarray(Wt.reshape(4, 128, 9 * 128)).astype(bf)

    w1bd = bd(inputs["w1_dw"])
    w2bd = bd(inputs["w2_dw"])

    pw1 = inputs["w1_pw"][:, :, 0, 0]              # [256, 512]
    pw2 = inputs["w2_pw"][:, :, 0, 0]
    pw1T = np.ascontiguousarray(pw1.T.reshape(4, 128, 256)).astype(bf)
    pw2T = np.ascontiguousarray(pw2.T.reshape(4, 128, 256)).astype(bf)

    gamma = float(inputs["gamma"][0])
    wvTg = np.ascontiguousarray(
        (inputs["wv"][:, :, 0, 0].T * gamma).reshape(2, 128, 256).astype(bf))
    wkT = np.ascontiguousarray(
        inputs["wk"][:, :, 0, 0].T.reshape(2, 128, 32)).astype(bf)
    wqT = np.ascontiguousarray(
        inputs["wq"][:, :, 0, 0].T.reshape(2, 128, 32)).astype(bf)

    def bn_fold(g, b_, mean, var, pw, b_dw, b_pw):
        s = g / np.sqrt(var + EPS)
        bc = pw @ b_dw + b_pw
        t = s * (bc - mean) + b_
        o = np.zeros((128, 4), f)
        o[:, 0], o[:, 1] = s[0:128], t[0:128]
        o[:, 2], o[:, 3] = s[128:256], t[128:256]
        return o

    bn1 = bn_fold(inputs["bn1_g"], inputs["bn1_b"], inputs["bn1_m"],
                  inputs["bn1_v"], pw1, inputs["b1_dw"], inputs["b1_pw"])
    bn2 = bn_fold(inputs["bn2_g"], inputs["bn2_b"], inputs["bn2_m"],
                  inputs["bn2_v"], pw2, inputs["b2_dw"], inputs["b2_pw"])

    bkq = np.zeros((128, 1), f)
    bkq[0:32, 0] = inputs["bk"]
    bkq[32:64, 0] = inputs["bq"]
    bvg = np.ascontiguousarray((gamma * inputs["bv"]).reshape(2, 128, 1).astype(f))

    vinit = np.zeros((128, 16 * VT), bf)
    for j in range(16):
        vinit[:, VT * j + 256] = 1.0

    return dict(w1bd=w1bd, w2bd=w2bd, pw1T=pw1T, pw2T=pw2T, wvT=wvTg,
                wkT=wkT, wqT=wqT, bn1=bn1, bn2=bn2, bkq=bkq, bvg=bvg,
                vinit=vinit)


def _prep_core(inputs, b, h):
    bf = ml_dtypes.bfloat16
    x1 = inputs["x1"][b]          # [256, 64, 64]
    x2 = inputs["x2"][b]
    sub = x1 - x2
    cat1 = np.concatenate([sub, x1], axis=0).reshape(4, 128, 64, 64)
    cat2 = np.concatenate([sub, x2], axis=0).reshape(4, 128, 64, 64)

    def pad_half(cc):
        buf = np.zeros((4, 128, SLOTS, 66), np.float32)
        if h == 0:
            buf[:, :, 1:34, 1:65] = cc[:, :, 0:33, :]
        else:
            buf[:, :, 0:33, 1:65] = cc[:, :, 31:64, :]
        catp = np.zeros((4, 128, CAT_F), bf)
        catp[:, :, OFF:OFF + SLOTS * PW] = buf.reshape(4, 128, -1)
        return catp

    x1h = np.ascontiguousarray(
        x1.reshape(256, N)[:, QH * h:QH * (h + 1)].reshape(2, 128, QH))
    return dict(cat1p=pad_half(cat1), cat2p=pad_half(cat2), x1h=x1h)


def kernel(**inputs):
    if "nc" not in _CACHE:
        _CACHE["nc"] = _build_nc()
    nc = _CACHE["nc"]

    inputs = {k: np.ascontiguousarray(np.asarray(v)) for k, v in inputs.items()}
    shared = _prep_shared(inputs)
    in_maps = []
    for core in range(8):
        b, h = core // 2, core % 2
        m = dict(shared)
        m.update(_prep_core(inputs, b, h))
        in_maps.append(m)

    res = run_bass_kernel_spmd(nc, in_maps, list(range(8)))
    out = np.empty((4, 256, N), np.float32)
    for core in range(8):
        b, h = core // 2, core % 2
        r = res.results[core]["out"]
        out[b, 0:128, QH * h:QH * (h + 1)] = r[0]
        out[b, 128:256, QH * h:QH * (h + 1)] = r[1]
    return out.reshape(B, C, H, W)

